# revision 3
# baseline (speedup 1.0000x reference)
"""Local (windowed) attention Trainium2 kernel.

Reference semantics (hardcoded, matching the nn.Module):
  q,k,v: [4, 16, 4096, 64] fp32. Windows of 128 along the sequence axis.
  Each query window attends to [prev window ; own window] (256 keys).
  Window -1 is PAD: k and v VALUES filled with -1.0 (not masked!).
  out = softmax(q*dh^-0.5 @ k_cat^T) @ v_cat.

Distribution: shard the fused (b*h)=64 axis across 8 NeuronCores, 8 rows
each; attention is window-local so there is no cross-core communication.

Kernel math per window (done fully on-chip):
  simT[j,i] = sum_e k[j,e] q[i,e]          (2 matmuls: j-chunks 128+128)
  expT      = exp(0.125*simT)               (ScalarE, reads PSUM)
  out_un[i, 0:65] = sum_j expT[j,i] * v_aug[j, :]   (v_aug has ones col 64)
  out[i,e]  = out_un[i,e] / out_un[i,64]
No max-subtraction: scores are ~N(0,1) here, exp is safe in fp32 and the
result matches jax.nn.softmax to ~1e-6 relative.
"""

import sys

sys.path.insert(0, "/opt/trn_rl_repo")

from contextlib import ExitStack

import numpy as np

import concourse.bass as bass
import concourse.tile as tile
from concourse import bacc, mybir
from concourse.bass_utils import run_bass_kernel_spmd
from concourse.masks import make_identity

B, H, N, DH = 4, 16, 4096, 64
WIN = 128
W = N // WIN  # 32 windows
NCORES = 8
BH = B * H  # 64
BH_PER_CORE = BH // NCORES  # 8
F32 = mybir.dt.float32
SCALE = DH ** -0.5  # 0.125
DMA_CHUNK = 8  # windows per bulk DMA chunk


def _build(nc):
    q = nc.dram_tensor("q", [BH_PER_CORE, N, DH], F32, kind="ExternalInput")
    k = nc.dram_tensor("k", [BH_PER_CORE, N, DH], F32, kind="ExternalInput")
    v = nc.dram_tensor("v", [BH_PER_CORE, N, DH], F32, kind="ExternalInput")
    out = nc.dram_tensor("out", [BH_PER_CORE, N, DH], F32, kind="ExternalOutput")

    with ExitStack() as ctx:
        tc = ctx.enter_context(tile.TileContext(nc))

        singles = ctx.enter_context(tc.tile_pool(name="singles", bufs=1))
        io = ctx.enter_context(tc.tile_pool(name="io", bufs=2))
        tsb = ctx.enter_context(tc.tile_pool(name="tsb", bufs=4))
        esb = ctx.enter_context(tc.tile_pool(name="esb", bufs=4))
        osb = ctx.enter_context(tc.tile_pool(name="osb", bufs=4))
        ps_tr = ctx.enter_context(tc.tile_pool(name="ps_tr", bufs=2, space="PSUM"))
        ps_sim = ctx.enter_context(tc.tile_pool(name="ps_sim", bufs=4, space="PSUM"))
        ps_out = ctx.enter_context(tc.tile_pool(name="ps_out", bufs=2, space="PSUM"))

        # Constants
        ident = singles.tile([128, 128], F32)
        make_identity(nc, ident)
        kneg = singles.tile([64, WIN], F32)   # transposed pad-K window (= -1)
        nc.vector.memset(kneg, -1.0)
        vneg = singles.tile([WIN, DH + 1], F32)  # pad-V window + ones col
        nc.vector.memset(vneg, -1.0)
        nc.vector.memset(vneg[:, DH:DH + 1], 1.0)

        for j in range(BH_PER_CORE):
            # Bulk loads for this bh row: SBUF layout [p=128, w, d]
            q_ap = q[j].rearrange("(w p) d -> p w d", p=WIN)
            k_ap = k[j].rearrange("(w p) d -> p w d", p=WIN)
            v_ap = v[j].rearrange("(w p) d -> p w d", p=WIN)
            o_ap = out[j].rearrange("(w p) d -> p w d", p=WIN)

            qt = io.tile([WIN, W, DH], F32, tag="qtile")
            kt = io.tile([WIN, W, DH], F32, tag="ktile")
            vt = io.tile([WIN, W, DH + 1], F32, tag="vtile")
            ot = io.tile([WIN, W, DH], F32, tag="otile")
            nc.vector.memset(vt[:, :, DH:DH + 1], 1.0)
            for c in range(0, W, DMA_CHUNK):
                s = slice(c, c + DMA_CHUNK)
                nc.sync.dma_start(out=qt[:, s, :], in_=q_ap[:, s, :])
                nc.sync.dma_start(out=kt[:, s, :], in_=k_ap[:, s, :])
                nc.sync.dma_start(out=vt[:, s, 0:DH], in_=v_ap[:, s, :])

            kT_prev = None
            for w in range(W):
                # Transpose q_w, k_w -> [64, 128] via TensorE
                qT_ps = ps_tr.tile([DH, WIN], F32, tag="trps")
                nc.tensor.transpose(qT_ps, qt[:, w, :], ident)
                qT = tsb.tile([DH, WIN], F32, tag="qT")
                nc.vector.tensor_copy(qT, qT_ps)

                kT_ps = ps_tr.tile([DH, WIN], F32, tag="trps")
                nc.tensor.transpose(kT_ps, kt[:, w, :], ident)
                kT = tsb.tile([DH, WIN], F32, tag="kT")
                nc.vector.tensor_copy(kT, kT_ps)

                kT0 = kneg if w == 0 else kT_prev
                v0 = vneg if w == 0 else vt[:, w - 1, :]

                # simT chunks: [j=128, i=128]
                sim0 = ps_sim.tile([WIN, WIN], F32, tag="sim")
                nc.tensor.matmul(sim0, lhsT=kT0, rhs=qT, start=True, stop=True)
                sim1 = ps_sim.tile([WIN, WIN], F32, tag="sim")
                nc.tensor.matmul(sim1, lhsT=kT, rhs=qT, start=True, stop=True)

                # expT = exp(scale * simT): PSUM -> SBUF on ScalarE
                exp0 = esb.tile([WIN, WIN], F32, tag="exp")
                nc.scalar.activation(exp0, sim0,
                                     mybir.ActivationFunctionType.Exp,
                                     scale=SCALE)
                exp1 = esb.tile([WIN, WIN], F32, tag="exp")
                nc.scalar.activation(exp1, sim1,
                                     mybir.ActivationFunctionType.Exp,
                                     scale=SCALE)

                # out_un[i, 0:65] accumulated over both j-chunks
                oun = ps_out.tile([WIN, DH + 1], F32, tag="oun")
                nc.tensor.matmul(oun, lhsT=exp0, rhs=v0, start=True, stop=False)
                nc.tensor.matmul(oun, lhsT=exp1, rhs=vt[:, w, :],
                                 start=False, stop=True)

                # normalize: out = out_un[:, :64] / out_un[:, 64]
                recip = osb.tile([WIN, 1], F32, tag="recip")
                nc.vector.reciprocal(recip, oun[:, DH:DH + 1])
                nc.vector.tensor_scalar_mul(ot[:, w, :], oun[:, 0:DH], recip)

                kT_prev = kT

            for c in range(0, W, DMA_CHUNK):
                s = slice(c, c + DMA_CHUNK)
                nc.sync.dma_start(out=o_ap[:, s, :], in_=ot[:, s, :])

    nc.finalize()
    return nc


_NC_CACHE = None


def _get_nc():
    global _NC_CACHE
    if _NC_CACHE is None:
        nc = bacc.Bacc("TRN2", target_bir_lowering=False, debug=False,
                       num_devices=NCORES)
        _NC_CACHE = _build(nc)
    return _NC_CACHE


def kernel(q, k, v, **_unused):
    q = np.ascontiguousarray(np.asarray(q, dtype=np.float32)).reshape(BH, N, DH)
    k = np.ascontiguousarray(np.asarray(k, dtype=np.float32)).reshape(BH, N, DH)
    v = np.ascontiguousarray(np.asarray(v, dtype=np.float32)).reshape(BH, N, DH)

    nc = _get_nc()
    in_maps = []
    for c in range(NCORES):
        s = slice(c * BH_PER_CORE, (c + 1) * BH_PER_CORE)
        in_maps.append({"q": q[s], "k": k[s], "v": v[s]})

    res = run_bass_kernel_spmd(nc, in_maps, list(range(NCORES))).results
    full = np.concatenate([res[c]["out"] for c in range(NCORES)], axis=0)
    return full.reshape(B, H, N, DH)


# revision 7
# speedup vs baseline: 2.4907x; 2.4907x over previous
"""Local (windowed) attention Trainium2 kernel — v2.

Reference semantics (hardcoded, matching the nn.Module):
  q,k,v: [4, 16, 4096, 64] fp32. Windows of 128 along the sequence axis.
  Each query window attends to [prev window ; own window] (256 keys).
  Window -1 is PAD: k and v VALUES filled with -1.0 (not masked!).
  out = softmax(q*dh^-0.5 @ k_cat^T) @ v_cat.

Distribution: shard the fused (b*h)=64 axis across 8 NeuronCores, 8 rows
each; attention is window-local so no cross-core communication.

v2 design notes (why it looks like this):
  * fp32 PE matmuls run at 1/4 rate (2 half-speed passes) and fp32
    LDWEIGHTS is ~2x. So: score matmuls use float32r (full rate when the
    moving dim is >=256) and the attention@V matmuls use bf16 weights
    (post-softmax weights tolerate bf16; FWL halves LDWEIGHTS).
  * Windows are paired: one matmul with stationary kT_w streams
    [qT_w | qT_{w+1}] (N=256), producing simT chunks (c1 of w | c0 of
    w+1) in one PSUM tile, which one Exp activation converts to the
    exact bf16 tile the AV matmuls slice as lhsT.
  * simT layout [keys, queries] everywhere: softmax denominator comes
    from a ones-column appended to V, normalization deferred to a DVE
    reciprocal + tensor_scalar after the AV matmul. No max-subtraction:
    scores here are ~N(0,1); exp is safe in fp32.
  * qT/kT live in contiguous per-bh [64, 4096] SBUF buffers so any
    window pair is a contiguous rhs slice; they are produced by PE
    pair-transposes ([128,256] PSUM -> 4 DVE half-copies).
"""

import sys

sys.path.insert(0, "/opt/trn_rl_repo")

from contextlib import ExitStack

import numpy as np

import concourse.bass as bass
import concourse.tile as tile
from concourse import bacc, mybir
from concourse.bass_utils import run_bass_kernel_spmd
from concourse.masks import make_identity

B, H, N, DH = 4, 16, 4096, 64
WIN = 128
W = N // WIN  # 32 windows
NCORES = 8
BH = B * H  # 64
BH_PER_CORE = BH // NCORES  # 8
F32 = mybir.dt.float32
F32R = mybir.dt.float32r
BF16 = mybir.dt.bfloat16
SCALE = DH ** -0.5  # 0.125
DMA_CHUNK = 8  # windows per bulk DMA chunk
EXPF = mybir.ActivationFunctionType.Exp


def _build(nc):
    q = nc.dram_tensor("q", [BH_PER_CORE, N, DH], F32, kind="ExternalInput")
    k = nc.dram_tensor("k", [BH_PER_CORE, N, DH], F32, kind="ExternalInput")
    v = nc.dram_tensor("v", [BH_PER_CORE, N, DH], F32, kind="ExternalInput")
    out = nc.dram_tensor("out", [BH_PER_CORE, N, DH], F32, kind="ExternalOutput")

    with ExitStack() as ctx:
        tc = ctx.enter_context(tile.TileContext(nc))

        singles = ctx.enter_context(tc.tile_pool(name="singles", bufs=1))
        io = ctx.enter_context(tc.tile_pool(name="io", bufs=2))
        tbh = ctx.enter_context(tc.tile_pool(name="tbh", bufs=2))
        esb = ctx.enter_context(tc.tile_pool(name="esb", bufs=4))
        osb = ctx.enter_context(tc.tile_pool(name="osb", bufs=4))
        ps_tr = ctx.enter_context(tc.tile_pool(name="ps_tr", bufs=2, space="PSUM"))
        ps_sim = ctx.enter_context(tc.tile_pool(name="ps_sim", bufs=3, space="PSUM"))
        ps_out = ctx.enter_context(tc.tile_pool(name="ps_out", bufs=3, space="PSUM"))

        # Constants
        ident = singles.tile([128, 128], F32)
        make_identity(nc, ident)
        kneg = singles.tile([64, WIN], F32R)   # transposed pad-K window (= -1)
        kneg_f32 = singles.tile([64, WIN], F32)
        nc.vector.memset(kneg_f32, -1.0)
        nc.vector.tensor_copy(kneg, kneg_f32)
        vneg = singles.tile([WIN, DH + 1], BF16)  # pad-V window + ones col
        nc.vector.memset(vneg, -1.0)
        nc.vector.memset(vneg[:, DH:DH + 1], 1.0)

        for j in range(BH_PER_CORE):
            q_ap = q[j].rearrange("(w p) d -> p w d", p=WIN)
            k_ap = k[j].rearrange("(w p) d -> p w d", p=WIN)
            v_ap = v[j].rearrange("(w p) d -> p w d", p=WIN)
            o_ap = out[j].rearrange("(w p) d -> p w d", p=WIN)

            qt = io.tile([WIN, W, DH], F32, tag="qtile")
            kt = io.tile([WIN, W, DH], F32, tag="ktile")
            vt = io.tile([WIN, W, DH], F32, tag="vtile")
            vbf = io.tile([WIN, W, DH + 1], BF16, tag="vbf")
            ot = io.tile([WIN, W, DH], F32, tag="otile")
            for c in range(0, W, DMA_CHUNK):
                s = slice(c, c + DMA_CHUNK)
                nc.sync.dma_start(out=qt[:, s, :], in_=q_ap[:, s, :])
                nc.sync.dma_start(out=kt[:, s, :], in_=k_ap[:, s, :])
                nc.sync.dma_start(out=vt[:, s, :], in_=v_ap[:, s, :])
                # convert v to bf16 and add the ones column (chunked so it
                # pipelines with the DMAs instead of waiting for all of them)
                nc.vector.memset(vbf[:, s, DH:DH + 1], 1.0)
                nc.vector.tensor_copy(vbf[:, s, 0:DH], vt[:, s, :])

            # Contiguous transposed views for the whole bh row
            qT = tbh.tile([DH, W * WIN], F32R, tag="qT")
            kT = tbh.tile([DH, W * WIN], F32R, tag="kT")
            for t in range(W // 2):  # window pairs (2t, 2t+1)
                trp = ps_tr.tile([128, 256], F32, tag="trp")
                nc.tensor.transpose(trp[:, 0:128], qt[:, 2 * t:2 * t + 2, :],
                                    ident)
                nc.tensor.transpose(trp[:, 128:256], kt[:, 2 * t:2 * t + 2, :],
                                    ident)
                c0 = 2 * t * WIN
                nc.vector.tensor_copy(qT[:, c0:c0 + WIN], trp[0:64, 0:128])
                nc.vector.tensor_copy(qT[:, c0 + WIN:c0 + 2 * WIN],
                                      trp[64:128, 0:128])
                nc.vector.tensor_copy(kT[:, c0:c0 + WIN], trp[0:64, 128:256])
                nc.vector.tensor_copy(kT[:, c0 + WIN:c0 + 2 * WIN],
                                      trp[64:128, 128:256])

            # Pad chunk for window 0: keys = all -1
            sim_pad = ps_sim.tile([WIN, 2 * WIN], F32, tag="sim")
            nc.tensor.matmul(sim_pad[:, 0:WIN], lhsT=kneg[:],
                             rhs=qT[:, 0:WIN], start=True, stop=True)
            exp_pad = esb.tile([WIN, 2 * WIN], BF16, tag="exp")
            nc.scalar.activation(exp_pad[:, 0:WIN], sim_pad[:, 0:WIN], EXPF,
                                 scale=SCALE)

            # Per-window sim pair matmuls + exp
            exps = []
            for w in range(W):
                simp = ps_sim.tile([WIN, 2 * WIN], F32, tag="sim")
                expp = esb.tile([WIN, 2 * WIN], BF16, tag="exp")
                ncols = 2 * WIN if w < W - 1 else WIN
                nc.tensor.matmul(simp[:, 0:ncols],
                                 lhsT=kT[:, w * WIN:(w + 1) * WIN],
                                 rhs=qT[:, w * WIN:w * WIN + ncols],
                                 start=True, stop=True)
                nc.scalar.activation(expp[:, 0:ncols], simp[:, 0:ncols], EXPF,
                                     scale=SCALE)
                exps.append(expp)

                # AV for window w (needs exp chunks c0 from w-1's pair, c1
                # from this pair)
                lhs0 = exp_pad[:, 0:WIN] if w == 0 else exps[w - 1][:, WIN:2 * WIN]
                rhs0 = vneg[:] if w == 0 else vbf[:, w - 1, :]
                oun = ps_out.tile([WIN, DH + 1], F32, tag="oun")
                nc.tensor.matmul(oun, lhsT=lhs0, rhs=rhs0, start=True,
                                 stop=False)
                nc.tensor.matmul(oun, lhsT=expp[:, 0:WIN], rhs=vbf[:, w, :],
                                 start=False, stop=True)

                recip = osb.tile([WIN, 1], F32, tag="recip")
                nc.vector.reciprocal(recip, oun[:, DH:DH + 1])
                nc.vector.tensor_scalar_mul(ot[:, w, :], oun[:, 0:DH], recip)

            for c in range(0, W, DMA_CHUNK):
                s = slice(c, c + DMA_CHUNK)
                nc.sync.dma_start(out=o_ap[:, s, :], in_=ot[:, s, :])

    nc.finalize()
    return nc


_NC_CACHE = None


def _get_nc():
    global _NC_CACHE
    if _NC_CACHE is None:
        nc = bacc.Bacc("TRN2", target_bir_lowering=False, debug=False,
                       num_devices=NCORES)
        _NC_CACHE = _build(nc)
    return _NC_CACHE


def kernel(q, k, v, **_unused):
    q = np.ascontiguousarray(np.asarray(q, dtype=np.float32)).reshape(BH, N, DH)
    k = np.ascontiguousarray(np.asarray(k, dtype=np.float32)).reshape(BH, N, DH)
    v = np.ascontiguousarray(np.asarray(v, dtype=np.float32)).reshape(BH, N, DH)

    nc = _get_nc()
    in_maps = []
    for c in range(NCORES):
        s = slice(c * BH_PER_CORE, (c + 1) * BH_PER_CORE)
        in_maps.append({"q": q[s], "k": k[s], "v": v[s]})

    res = run_bass_kernel_spmd(nc, in_maps, list(range(NCORES))).results
    full = np.concatenate([res[c]["out"] for c in range(NCORES)], axis=0)
    return full.reshape(B, H, N, DH)


# revision 12
# speedup vs baseline: 3.2302x; 1.2969x over previous
"""Local (windowed) attention Trainium2 kernel — v4.

Reference semantics (hardcoded, matching the nn.Module):
  q,k,v: [4, 16, 4096, 64] fp32. Windows of 128 along the sequence axis.
  Each query window attends to [prev window ; own window] (256 keys).
  Window -1 is PAD: k and v VALUES filled with -1.0 (not masked!).
  out = softmax(q*dh^-0.5 @ k_cat^T) @ v_cat.

Distribution: shard the fused (b*h)=64 axis across 8 NeuronCores, 8 rows
each; attention is window-local so no cross-core communication.

v4 design (why it looks like this):
  * All on-chip matmul operands are fp16 at base_partition 0: fp32 PE
    matmuls run at 1/4 rate, and operands at base_partition 64 crash the
    device at scale, so the stacked-pair DMA-transpose layout of v3 is
    out. Scores accumulate in fp32 PSUM; end-to-end rel err ~1e-3.
  * q/k are transposed per-window on TensorE (fp16 transpose is full
    rate) into a [64, 512] PSUM staging tile (4 windows), then one DVE
    copy per 4 windows lands them in contiguous [64, 4096] qT/kT.
  * sim matmuls are window-paired: stationary kT_w streams
    [qT_w | qT_{w+1}] (N=256), producing chunks (c1 of w | c0 of w+1);
    two such pair-results share a [128,512] PSUM bank so one Exp
    activation covers 4 chunks (amortizes ACT overhead).
  * simT layout [keys, queries]: softmax denominator comes from a
    ones-column appended to V; normalization is deferred and batched
    per-bh (strided extraction + one reciprocal + one broadcast mul).
  * No max-subtraction: scores are ~N(0,1) here; exp is safe in fp32.
"""

import sys

sys.path.insert(0, "/opt/trn_rl_repo")

from contextlib import ExitStack

import numpy as np

import concourse.bass as bass
import concourse.tile as tile
from concourse import bacc, mybir
from concourse.bass_utils import run_bass_kernel_spmd
from concourse.masks import make_identity

B, H, N, DH = 4, 16, 4096, 64
WIN = 128
W = N // WIN  # 32 windows
NCORES = 8
BH = B * H
BH_PER_CORE = BH // NCORES  # 8
F32 = mybir.dt.float32
F16 = mybir.dt.float16
SCALE = DH ** -0.5  # 0.125
DMA_CHUNK = 8
EXPF = mybir.ActivationFunctionType.Exp


def _build(nc):
    q = nc.dram_tensor("q", [BH_PER_CORE, N, DH], F32, kind="ExternalInput")
    k = nc.dram_tensor("k", [BH_PER_CORE, N, DH], F32, kind="ExternalInput")
    v = nc.dram_tensor("v", [BH_PER_CORE, N, DH], F32, kind="ExternalInput")
    out = nc.dram_tensor("out", [BH_PER_CORE, N, DH], F32, kind="ExternalOutput")

    with ExitStack() as ctx:
        tc = ctx.enter_context(tile.TileContext(nc))

        singles = ctx.enter_context(tc.tile_pool(name="singles", bufs=1))
        io = ctx.enter_context(tc.tile_pool(name="io", bufs=2))
        tbh = ctx.enter_context(tc.tile_pool(name="tbh", bufs=2))
        esb = ctx.enter_context(tc.tile_pool(name="esb", bufs=4))
        nsb = ctx.enter_context(tc.tile_pool(name="nsb", bufs=2))
        ps_tr = ctx.enter_context(tc.tile_pool(name="ps_tr", bufs=3, space="PSUM"))
        ps_sim = ctx.enter_context(tc.tile_pool(name="ps_sim", bufs=3, space="PSUM"))
        ps_out = ctx.enter_context(tc.tile_pool(name="ps_out", bufs=2, space="PSUM"))

        ident = singles.tile([128, 128], F16)
        make_identity(nc, ident)
        kneg = singles.tile([64, WIN], F16)
        nc.vector.memset(kneg, -1.0)
        vneg = singles.tile([WIN, DH + 1], F16)
        nc.vector.memset(vneg, -1.0)
        nc.vector.memset(vneg[:, DH:DH + 1], 1.0)

        for j in range(BH_PER_CORE):
            q_ap = q[j].rearrange("(w p) d -> p w d", p=WIN)
            k_ap = k[j].rearrange("(w p) d -> p w d", p=WIN)
            v_ap = v[j].rearrange("(w p) d -> p w d", p=WIN)
            o_ap = out[j].rearrange("(w p) d -> p w d", p=WIN)

            qt = io.tile([WIN, W, DH], F32, tag="qtile")
            kt = io.tile([WIN, W, DH], F32, tag="ktile")
            vt = io.tile([WIN, W, DH], F32, tag="vtile")
            qh = io.tile([WIN, W, DH], F16, tag="qh")
            kh = io.tile([WIN, W, DH], F16, tag="kh")
            vh = io.tile([WIN, W, DH + 1], F16, tag="vh")
            ot = io.tile([WIN, W, DH], F32, tag="otile")
            for c in range(0, W, DMA_CHUNK):
                s = slice(c, c + DMA_CHUNK)
                nc.sync.dma_start(out=qt[:, s, :], in_=q_ap[:, s, :])
                nc.sync.dma_start(out=kt[:, s, :], in_=k_ap[:, s, :])
                nc.sync.dma_start(out=vt[:, s, :], in_=v_ap[:, s, :])
                nc.vector.tensor_copy(qh[:, s, :], qt[:, s, :])
                nc.vector.tensor_copy(kh[:, s, :], kt[:, s, :])
                nc.vector.memset(vh[:, s, DH:DH + 1], 1.0)
                nc.vector.tensor_copy(vh[:, s, 0:DH], vt[:, s, :])

            # Contiguous transposed views [dh=64, W*128] via per-window
            # TensorE transposes staged 4-at-a-time in [64, 512] PSUM.
            qT = tbh.tile([DH, W * WIN], F16, tag="qT")
            kT = tbh.tile([DH, W * WIN], F16, tag="kT")
            for c in range(0, W, 4):
                trq = ps_tr.tile([DH, 4 * WIN], F16, tag="trp")
                trk = ps_tr.tile([DH, 4 * WIN], F16, tag="trp")
                for m in range(4):
                    nc.tensor.transpose(trq[:, m * WIN:(m + 1) * WIN],
                                        qh[:, c + m, :], ident)
                    nc.tensor.transpose(trk[:, m * WIN:(m + 1) * WIN],
                                        kh[:, c + m, :], ident)
                nc.vector.tensor_copy(qT[:, c * WIN:(c + 4) * WIN], trq)
                nc.vector.tensor_copy(kT[:, c * WIN:(c + 4) * WIN], trk)

            denb = nsb.tile([WIN, W], F32, tag="denb")
            rden = nsb.tile([WIN, W], F32, tag="rden")

            for g in range(W // 2):  # 2 windows per iteration
                w0, w1 = 2 * g, 2 * g + 1
                # simT chunk layout in one PSUM bank:
                # cols 0:128   = kT_{w0-1}.T qT_{w0}        (c0 of w0)
                # cols 128:384 = kT_{w0}.T [qT_{w0}|qT_{w1}] (c1 w0, c0 w1)
                # cols 384:512 = kT_{w1}.T qT_{w1}          (c1 of w1)
                simg = ps_sim.tile([WIN, 4 * WIN], F32, tag="sim")
                lhs0 = kneg[:] if g == 0 else kT[:, (w0 - 1) * WIN:w0 * WIN]
                nc.tensor.matmul(simg[:, 0:128], lhsT=lhs0,
                                 rhs=qT[:, w0 * WIN:(w0 + 1) * WIN],
                                 start=True, stop=True)
                nc.tensor.matmul(simg[:, 128:384],
                                 lhsT=kT[:, w0 * WIN:(w0 + 1) * WIN],
                                 rhs=qT[:, w0 * WIN:(w0 + 2) * WIN],
                                 start=True, stop=True)
                nc.tensor.matmul(simg[:, 384:512],
                                 lhsT=kT[:, w1 * WIN:(w1 + 1) * WIN],
                                 rhs=qT[:, w1 * WIN:(w1 + 1) * WIN],
                                 start=True, stop=True)

                expg = esb.tile([WIN, 4 * WIN], F16, tag="exp")
                nc.scalar.activation(expg, simg, EXPF, scale=SCALE)

                # AV: w0 chunks are exp cols (0:128, 128:256);
                # w1 chunks are (256:384, 384:512)
                oun = ps_out.tile([WIN, 2 * (DH + 1)], F32, tag="oun")
                nc.tensor.matmul(oun[:, 0:DH + 1], lhsT=expg[:, 0:128],
                                 rhs=vneg[:] if g == 0 else vh[:, w0 - 1, :],
                                 start=True, stop=False)
                nc.tensor.matmul(oun[:, 0:DH + 1], lhsT=expg[:, 128:256],
                                 rhs=vh[:, w0, :], start=False, stop=True)
                nc.tensor.matmul(oun[:, DH + 1:], lhsT=expg[:, 256:384],
                                 rhs=vh[:, w0, :], start=True, stop=False)
                nc.tensor.matmul(oun[:, DH + 1:], lhsT=expg[:, 384:512],
                                 rhs=vh[:, w1, :], start=False, stop=True)

                nc.vector.tensor_copy(
                    ot[:, w0:w1 + 1, :],
                    oun.rearrange("p (m c) -> p m c", m=2)[:, :, 0:DH])
                nc.vector.tensor_copy(
                    denb[:, w0:w1 + 1],
                    oun.rearrange("p (m c) -> p m c", m=2)[:, :, DH])

            nc.vector.reciprocal(rden, denb)
            rd = rden[:]
            bcast = bass.AP(tensor=rd.tensor, offset=rd.offset,
                            ap=[rd.ap[0], rd.ap[1], [0, DH]])
            nc.vector.tensor_mul(ot[:], ot[:], bcast)

            for c in range(0, W, DMA_CHUNK):
                s = slice(c, c + DMA_CHUNK)
                nc.sync.dma_start(out=o_ap[:, s, :], in_=ot[:, s, :])

    nc.finalize()
    return nc


_NC_CACHE = None


def _get_nc():
    global _NC_CACHE
    if _NC_CACHE is None:
        nc = bacc.Bacc("TRN2", target_bir_lowering=False, debug=False,
                       num_devices=NCORES)
        _NC_CACHE = _build(nc)
    return _NC_CACHE


def kernel(q, k, v, **_unused):
    q = np.ascontiguousarray(np.asarray(q, dtype=np.float32)).reshape(BH, N, DH)
    k = np.ascontiguousarray(np.asarray(k, dtype=np.float32)).reshape(BH, N, DH)
    v = np.ascontiguousarray(np.asarray(v, dtype=np.float32)).reshape(BH, N, DH)

    nc = _get_nc()
    in_maps = []
    for c in range(NCORES):
        s = slice(c * BH_PER_CORE, (c + 1) * BH_PER_CORE)
        in_maps.append({"q": q[s], "k": k[s], "v": v[s]})

    res = run_bass_kernel_spmd(nc, in_maps, list(range(NCORES))).results
    full = np.concatenate([res[c]["out"] for c in range(NCORES)], axis=0)
    return full.reshape(B, H, N, DH)
